# revision 13
# baseline (speedup 1.0000x reference)
"""GF(2) linear block encoder c = (b @ G) mod 2 on 8 TRN2 NeuronCores.

Strategy:
  - Data-parallel: shard b rows (32768 -> 8 x 4096), replicate G.
  - Bits {0,1} are exact in fp8-e4m3 and products accumulate exactly in
    fp32 PSUM, so the GF(2) matmul is an fp8 DoubleRow matmul (K=256 per
    MM) at 2x bf16 throughput -- the PE floor for this shape (~110us).
  - Output is uint16 bits (ACT casts PSUM fp32 -> uint16, DVE ands with
    1) upcast to int32 on the host: 2x less output HBM traffic, which
    removes the output-DMA tail.
  - DMA queues only start moving ~9us in, so the PE is pre-warmed with
    dummy matmuls (p-state ramp) and tiles 0-1 are computed DMA-paced:
    G arrives as 16 quarter-tiles and b chunk 0 as 4 k-pieces, queued in
    exact consumption order (nt-outer, kp-inner) so the first matmul
    needs only 192 KiB instead of 6 MiB.
"""

import sys

import numpy as np

if "/opt/trn_rl_repo" not in sys.path:
    sys.path.insert(0, "/opt/trn_rl_repo")

import ml_dtypes

B_ROWS = 32768
K_MSG = 1024
N_CODE = 2048
NCORES = 8
M = B_ROWS // NCORES  # 4096 rows per core
KS = K_MSG // 128     # 8 k-subtiles of 128
KP = KS // 2          # 4 DoubleRow k-pair steps (K=256 each)
MT = M // 128         # 32 m-tiles
NT = N_CODE // 512    # 4 n-chunks (one PSUM bank each)
MC = 16               # b DMA chunks along m (2 m-tiles each)
MCW = M // MC         # 256 rows per chunk

F8 = ml_dtypes.float8_e4m3

_NC_CACHE = None


def _build_bass():
    import concourse.bacc as bacc
    import concourse.mybir as mybir
    from concourse import tile

    nc = bacc.Bacc("TRN2", target_bir_lowering=False, debug=False)

    # bt[p, c, s, j] = b bit for row m = c*MCW + j, k = s*128 + p
    bt = nc.dram_tensor("bt", [128, MC, KS, MCW], mybir.dt.float8e4, kind="ExternalInput")
    g = nc.dram_tensor("g", [128, KS, N_CODE], mybir.dt.float8e4, kind="ExternalInput")
    c = nc.dram_tensor("c", [M, N_CODE], mybir.dt.uint16, kind="ExternalOutput")

    dr = mybir.MatmulPerfMode.DoubleRow

    with tile.TileContext(nc) as tc:
        with (
            tc.tile_pool(name="persist", bufs=1) as persist,
            tc.tile_pool(name="psum", bufs=2, space="PSUM") as psum_pool,
            tc.tile_pool(name="mids", bufs=6) as mids,
        ):
            g_tiles = [
                persist.tile([128, 2, N_CODE], mybir.dt.float8e4, name=f"gt{kp}", tag=f"g{kp}")
                for kp in range(KP)
            ]
            b_tiles = [
                persist.tile([128, KS, MCW], mybir.dt.float8e4, name=f"btile{mc}", tag=f"b{mc}")
                for mc in range(MC)
            ]

            def load_b(mc, eng):
                eng.dma_start(out=b_tiles[mc], in_=bt[:, mc, :, :])

            def load_b_kp(mc, kp, eng):
                eng.dma_start(
                    out=b_tiles[mc][:, 2 * kp : 2 * kp + 2, :],
                    in_=bt[:, mc, 2 * kp : 2 * kp + 2, :],
                )

            def load_g_q(kp, q, eng):
                eng.dma_start(
                    out=g_tiles[kp][:, :, q * 512 : (q + 1) * 512],
                    in_=g[:, 2 * kp : 2 * kp + 2, q * 512 : (q + 1) * 512],
                )

            # Input DMAs in exact consumption order of the DMA-paced tiles
            # 0-1 (nt-outer, kp-inner), alternating the two HWDGE queues so
            # the 16 engines drain both. b chunk 0 arrives as 4 k-pieces.
            load_b_kp(0, 0, nc.sync)      # 64 KiB  (first matmul)
            load_g_q(0, 0, nc.scalar)     # 128 KiB (first matmul)
            load_b_kp(0, 1, nc.sync)
            load_g_q(1, 0, nc.scalar)
            load_b_kp(0, 2, nc.sync)
            load_g_q(2, 0, nc.scalar)
            load_b_kp(0, 3, nc.sync)
            load_g_q(3, 0, nc.scalar)
            eng_rr = [nc.sync, nc.scalar]
            i = 0
            for q in range(1, NT):
                for kp in range(KP):
                    load_g_q(kp, q, eng_rr[i % 2])
                    i += 1
            for mc in range(1, MC):
                load_b(mc, eng_rr[mc % 2])

            # PE p-state pre-warm with dummy matmuls while queues start up
            zb = persist.tile([128, 2, 128], mybir.dt.float8e4, name="zwarm")
            nc.vector.memset(zb, 0)
            ps_warm = psum_pool.tile([128, N_CODE], mybir.dt.float32, name="ps")
            for w in range(18):
                nc.tensor.matmul(
                    ps_warm[:, 0:128], zb, zb, start=True, stop=True, perf_mode=dr
                )

            # output viewed per m-tile: m = mt*128 + p
            c_view = c.rearrange("(mt p) n -> mt p n", p=128)

            # out-DMA queues: early tiles on SWDGE (input lines are queued
            # ahead of outputs on the HWDGE rings and would delay them,
            # stalling mid-tile reuse); later tiles rotate over all three
            out_eng = [nc.gpsimd] * 8 + [
                (nc.gpsimd, nc.sync, nc.scalar)[i % 3] for i in range(MT - 8)
            ]

            def mm(ps, mt, kp, nt):
                mc, j = mt // 2, mt % 2
                nc.tensor.matmul(
                    ps[:, nt * 512 : (nt + 1) * 512],
                    b_tiles[mc][:, 2 * kp : 2 * kp + 2, j * 128 : (j + 1) * 128],
                    g_tiles[kp][:, :, nt * 512 : (nt + 1) * 512],
                    start=(kp == 0),
                    stop=(kp == KP - 1),
                    perf_mode=dr,
                )

            def extract(ps, mid, n0, n1, eng):
                nc.scalar.activation(
                    mid[:, n0:n1], ps[:, n0:n1], mybir.ActivationFunctionType.Copy
                )
                eng.tensor_scalar(
                    out=mid[:, n0:n1],
                    in0=mid[:, n0:n1],
                    scalar1=1,
                    scalar2=None,
                    op0=mybir.AluOpType.bitwise_and,
                )

            # tiles 0,1: DMA-paced, nt-outer kp-inner, interleaved across
            # the pair (both use b chunk 0) to match input arrival order
            ps_pair = [
                psum_pool.tile([128, N_CODE], mybir.dt.float32, name="ps")
                for _ in range(2)
            ]
            for nt in range(NT):
                for kp in range(KP):
                    mm(ps_pair[0], 0, kp, nt)
                    mm(ps_pair[1], 1, kp, nt)
            for i in range(2):
                mid = mids.tile([128, N_CODE], mybir.dt.uint16)
                extract(ps_pair[i], mid, 0, N_CODE, nc.vector)
                out_eng[i].dma_start(out=c_view[i], in_=mid)

            for mt in range(2, MT):
                ps = psum_pool.tile([128, N_CODE], mybir.dt.float32, name="ps")
                if mt < MT - 1:
                    for kp in range(KP):
                        for nt in range(NT):
                            mm(ps, mt, kp, nt)
                    mid = mids.tile([128, N_CODE], mybir.dt.uint16)
                    extract(ps, mid, 0, N_CODE, nc.vector)
                    out_eng[mt].dma_start(out=c_view[mt], in_=mid)
                else:
                    # last tile: nt-outer, extract + stream out each 512-col
                    # quarter while the PE finishes the later quarters
                    mid = mids.tile([128, N_CODE], mybir.dt.uint16)
                    for nt in range(NT):
                        for kp in range(KP):
                            mm(ps, mt, kp, nt)
                        extract(ps, mid, nt * 512, (nt + 1) * 512, nc.vector)
                        out_eng[mt].dma_start(
                            out=c_view[mt][:, nt * 512 : (nt + 1) * 512],
                            in_=mid[:, nt * 512 : (nt + 1) * 512],
                        )

    nc.finalize()
    return nc


def _get_nc():
    global _NC_CACHE
    if _NC_CACHE is None:
        _NC_CACHE = _build_bass()
    return _NC_CACHE


def _pack_inputs(b, G):
    b8 = np.asarray(b).astype(np.uint8)
    G8 = np.asarray(G).astype(np.uint8)
    # g[p, s, n], k = s*128 + p
    g_f8 = G8.reshape(KS, 128, N_CODE).transpose(1, 0, 2).astype(F8, order="C")
    bts = []
    for core in range(NCORES):
        sh = b8[core * M : (core + 1) * M]  # [M, K]
        # bt[p, c, s, j]: m = c*MCW + j, k = s*128 + p
        btc = sh.reshape(MC, MCW, KS, 128).transpose(3, 0, 2, 1)
        bts.append(btc.astype(F8, order="C"))
    return bts, g_f8


def kernel(b, G, trace=False, **run_kwargs):
    from concourse.bass_utils import run_bass_kernel_spmd

    nc = _get_nc()
    bts, g_f8 = _pack_inputs(b, G)
    in_maps = [{"bt": bts[i], "g": g_f8} for i in range(NCORES)]
    res = run_bass_kernel_spmd(
        nc, in_maps, core_ids=list(range(NCORES)), trace=trace, **run_kwargs
    )
    out = np.concatenate([res.results[i]["c"] for i in range(NCORES)], axis=0)
    out = out.astype(np.int32)
    if trace:
        kernel.last_results = res
    return out


kernel.last_results = None


# revision 14
# speedup vs baseline: 1.0374x; 1.0374x over previous
"""GF(2) linear block encoder c = (b @ G) mod 2 on 8 TRN2 NeuronCores.

Strategy:
  - Data-parallel: shard b rows (32768 -> 8 x 4096), replicate G.
  - Bits {0,1} are exact in fp8-e4m3 and products accumulate exactly in
    fp32 PSUM, so the GF(2) matmul is an fp8 DoubleRow matmul (K=256 per
    MM) at 2x bf16 throughput -- the PE floor for this shape (~110us).
  - Output is uint16 bits (ACT casts PSUM fp32 -> uint16, DVE ands with
    1) upcast to int32 on the host: 2x less output HBM traffic, which
    removes the output-DMA tail.
  - DMA queues only start moving ~9us in, so the PE is pre-warmed with
    dummy matmuls (p-state ramp) and tiles 0-1 are computed DMA-paced:
    G arrives as 16 quarter-tiles and b chunk 0 as 4 k-pieces, queued in
    exact consumption order (nt-outer, kp-inner) so the first matmul
    needs only 192 KiB instead of 6 MiB.
"""

import sys

import numpy as np

if "/opt/trn_rl_repo" not in sys.path:
    sys.path.insert(0, "/opt/trn_rl_repo")

import ml_dtypes

B_ROWS = 32768
K_MSG = 1024
N_CODE = 2048
NCORES = 8
M = B_ROWS // NCORES  # 4096 rows per core
KS = K_MSG // 128     # 8 k-subtiles of 128
KP = KS // 2          # 4 DoubleRow k-pair steps (K=256 each)
MT = M // 128         # 32 m-tiles
NT = N_CODE // 512    # 4 n-chunks (one PSUM bank each)
MC = 16               # b DMA chunks along m (2 m-tiles each)
MCW = M // MC         # 256 rows per chunk

F8 = ml_dtypes.float8_e4m3

_NC_CACHE = None


def _build_bass():
    import concourse.bacc as bacc
    import concourse.mybir as mybir
    from concourse import tile

    nc = bacc.Bacc("TRN2", target_bir_lowering=False, debug=False)

    # bt[p, c, s, j] = b bit for row m = c*MCW + j, k = s*128 + p
    bt = nc.dram_tensor("bt", [128, MC, KS, MCW], mybir.dt.float8e4, kind="ExternalInput")
    g = nc.dram_tensor("g", [128, KS, N_CODE], mybir.dt.float8e4, kind="ExternalInput")
    c = nc.dram_tensor("c", [M, N_CODE], mybir.dt.uint16, kind="ExternalOutput")

    dr = mybir.MatmulPerfMode.DoubleRow

    with tile.TileContext(nc) as tc:
        with (
            tc.tile_pool(name="persist", bufs=1) as persist,
            tc.tile_pool(name="psum", bufs=2, space="PSUM") as psum_pool,
            tc.tile_pool(name="mids", bufs=6) as mids,
        ):
            g_tiles = [
                persist.tile([128, 2, N_CODE], mybir.dt.float8e4, name=f"gt{kp}", tag=f"g{kp}")
                for kp in range(KP)
            ]
            b_tiles = [
                persist.tile([128, KS, MCW], mybir.dt.float8e4, name=f"btile{mc}", tag=f"b{mc}")
                for mc in range(MC)
            ]

            def load_b(mc, eng):
                eng.dma_start(out=b_tiles[mc], in_=bt[:, mc, :, :])

            def load_b_kp(mc, kp, eng):
                eng.dma_start(
                    out=b_tiles[mc][:, 2 * kp : 2 * kp + 2, :],
                    in_=bt[:, mc, 2 * kp : 2 * kp + 2, :],
                )

            def load_g_q(kp, q, eng):
                eng.dma_start(
                    out=g_tiles[kp][:, :, q * 512 : (q + 1) * 512],
                    in_=g[:, 2 * kp : 2 * kp + 2, q * 512 : (q + 1) * 512],
                )

            # Input DMAs in exact consumption order of the DMA-paced tile
            # 0 (kp-outer, nt-inner): G quarters kp-major, b chunk 0 as 4
            # k-pieces interleaved, then b1..b3 immediately (tiles 2-7
            # consume them at PE pace), then the rest.
            eng_rr = [nc.sync, nc.scalar]
            i = 0
            for kp in range(KP):
                load_b_kp(0, kp, eng_rr[i % 2]); i += 1
                for q in range(NT):
                    load_g_q(kp, q, eng_rr[i % 2]); i += 1
            for mc in range(1, MC):
                load_b(mc, eng_rr[i % 2]); i += 1

            # PE p-state pre-warm with dummy matmuls while queues start up
            zb = persist.tile([128, 2, 128], mybir.dt.float8e4, name="zwarm")
            nc.vector.memset(zb, 0)
            ps_warm = psum_pool.tile([128, N_CODE], mybir.dt.float32, name="ps")
            for w in range(18):
                nc.tensor.matmul(
                    ps_warm[:, 0:128], zb, zb, start=True, stop=True, perf_mode=dr
                )

            # output viewed per m-tile: m = mt*128 + p
            c_view = c.rearrange("(mt p) n -> mt p n", p=128)

            # out-DMA queues: early tiles on SWDGE (input lines are queued
            # ahead of outputs on the HWDGE rings and would delay them,
            # stalling mid-tile reuse); later tiles rotate over all three
            out_eng = [nc.gpsimd] * 8 + [
                (nc.gpsimd, nc.sync, nc.scalar)[i % 3] for i in range(MT - 8)
            ]

            def mm(ps, mt, kp, nt):
                mc, j = mt // 2, mt % 2
                nc.tensor.matmul(
                    ps[:, nt * 512 : (nt + 1) * 512],
                    b_tiles[mc][:, 2 * kp : 2 * kp + 2, j * 128 : (j + 1) * 128],
                    g_tiles[kp][:, :, nt * 512 : (nt + 1) * 512],
                    start=(kp == 0),
                    stop=(kp == KP - 1),
                    perf_mode=dr,
                )

            def extract(ps, mid, n0, n1, eng):
                nc.scalar.activation(
                    mid[:, n0:n1], ps[:, n0:n1], mybir.ActivationFunctionType.Copy
                )
                eng.tensor_scalar(
                    out=mid[:, n0:n1],
                    in0=mid[:, n0:n1],
                    scalar1=1,
                    scalar2=None,
                    op0=mybir.AluOpType.bitwise_and,
                )

            for mt in range(0, MT):
                ps = psum_pool.tile([128, N_CODE], mybir.dt.float32, name="ps")
                if mt < MT - 1:
                    for kp in range(KP):
                        for nt in range(NT):
                            mm(ps, mt, kp, nt)
                    mid = mids.tile([128, N_CODE], mybir.dt.uint16)
                    extract(ps, mid, 0, N_CODE, nc.vector)
                    out_eng[mt].dma_start(out=c_view[mt], in_=mid)
                else:
                    # last tile: nt-outer over four SEPARATE per-bank PSUM
                    # tiles (no false whole-tile WAR deps), extracting and
                    # streaming out each 512-col quarter while the PE
                    # finishes the later quarters
                    mid = mids.tile([128, N_CODE], mybir.dt.uint16)
                    for nt in range(NT):
                        psq = psum_pool.tile([128, 512], mybir.dt.float32, name="ps")
                        for kp in range(KP):
                            mc, j = mt // 2, mt % 2
                            nc.tensor.matmul(
                                psq,
                                b_tiles[mc][:, 2 * kp : 2 * kp + 2, j * 128 : (j + 1) * 128],
                                g_tiles[kp][:, :, nt * 512 : (nt + 1) * 512],
                                start=(kp == 0),
                                stop=(kp == KP - 1),
                                perf_mode=dr,
                            )
                        nc.scalar.activation(
                            mid[:, nt * 512 : (nt + 1) * 512],
                            psq,
                            mybir.ActivationFunctionType.Copy,
                        )
                        nc.vector.tensor_scalar(
                            out=mid[:, nt * 512 : (nt + 1) * 512],
                            in0=mid[:, nt * 512 : (nt + 1) * 512],
                            scalar1=1,
                            scalar2=None,
                            op0=mybir.AluOpType.bitwise_and,
                        )
                        out_eng[mt].dma_start(
                            out=c_view[mt][:, nt * 512 : (nt + 1) * 512],
                            in_=mid[:, nt * 512 : (nt + 1) * 512],
                        )

    nc.finalize()
    return nc


def _get_nc():
    global _NC_CACHE
    if _NC_CACHE is None:
        _NC_CACHE = _build_bass()
    return _NC_CACHE


def _pack_inputs(b, G):
    b8 = np.asarray(b).astype(np.uint8)
    G8 = np.asarray(G).astype(np.uint8)
    # g[p, s, n], k = s*128 + p
    g_f8 = G8.reshape(KS, 128, N_CODE).transpose(1, 0, 2).astype(F8, order="C")
    bts = []
    for core in range(NCORES):
        sh = b8[core * M : (core + 1) * M]  # [M, K]
        # bt[p, c, s, j]: m = c*MCW + j, k = s*128 + p
        btc = sh.reshape(MC, MCW, KS, 128).transpose(3, 0, 2, 1)
        bts.append(btc.astype(F8, order="C"))
    return bts, g_f8


def kernel(b, G, trace=False, **run_kwargs):
    from concourse.bass_utils import run_bass_kernel_spmd

    nc = _get_nc()
    bts, g_f8 = _pack_inputs(b, G)
    in_maps = [{"bt": bts[i], "g": g_f8} for i in range(NCORES)]
    res = run_bass_kernel_spmd(
        nc, in_maps, core_ids=list(range(NCORES)), trace=trace, **run_kwargs
    )
    out = np.concatenate([res.results[i]["c"] for i in range(NCORES)], axis=0)
    out = out.astype(np.int32)
    if trace:
        kernel.last_results = res
    return out


kernel.last_results = None


# revision 15
# speedup vs baseline: 1.0470x; 1.0092x over previous
"""GF(2) linear block encoder c = (b @ G) mod 2 on 8 TRN2 NeuronCores.

Strategy:
  - Data-parallel: shard b rows (32768 -> 8 x 4096), replicate G.
  - Bits {0,1} are exact in fp8-e4m3 and products accumulate exactly in
    fp32 PSUM, so the GF(2) matmul is an fp8 DoubleRow matmul (K=256 per
    MM) at 2x bf16 throughput -- the PE floor for this shape (~110us).
  - Output is uint16 bits (ACT casts PSUM fp32 -> uint16, DVE ands with
    1), upcast to int32 on the host: 2x less output HBM traffic.
  - dma_start costs ~0.7us of issuing-sequencer time, so pushes are
    budgeted: inputs ride the sync + gpsimd queues only (scalar must
    reach its extraction COPYs immediately or PSUM backpressure stalls
    the PE), b arrives in 4-chunk groups (8KB/partition lines), and G
    is ordered g0-halves, g1..g3 so tile 0 can run kp-outer DMA-paced.
  - PE p-state is pre-warmed with dummy matmuls during the ~9us DMA
    queue startup; the last tile extracts per 512-col PSUM bank so the
    tail is one quarter-extract + one 64KiB DMA.
"""

import sys

import numpy as np

if "/opt/trn_rl_repo" not in sys.path:
    sys.path.insert(0, "/opt/trn_rl_repo")

import ml_dtypes

B_ROWS = 32768
K_MSG = 1024
N_CODE = 2048
NCORES = 8
M = B_ROWS // NCORES  # 4096 rows per core
KS = K_MSG // 128     # 8 k-subtiles of 128
KP = KS // 2          # 4 DoubleRow k-pair steps (K=256 each)
MT = M // 128         # 32 m-tiles
NT = N_CODE // 512    # 4 n-chunks (one PSUM bank each)
MC = 16               # b chunks along m (2 m-tiles each)
MCW = M // MC         # 256 rows per chunk
BG = 4                # b chunks per DMA group
NBG = MC // BG        # 4 groups

F8 = ml_dtypes.float8_e4m3

_NC_CACHE = None


def _build_bass():
    import concourse.bacc as bacc
    import concourse.mybir as mybir
    from concourse import tile

    nc = bacc.Bacc("TRN2", target_bir_lowering=False, debug=False)

    # bt[p, c, s, j] = b bit for row m = c*MCW + j, k = s*128 + p
    bt = nc.dram_tensor("bt", [128, MC, KS, MCW], mybir.dt.float8e4, kind="ExternalInput")
    g = nc.dram_tensor("g", [128, KS, N_CODE], mybir.dt.float8e4, kind="ExternalInput")
    c = nc.dram_tensor("c", [M, N_CODE], mybir.dt.uint16, kind="ExternalOutput")

    dr = mybir.MatmulPerfMode.DoubleRow
    NH = N_CODE // 2

    with tile.TileContext(nc) as tc:
        with (
            tc.tile_pool(name="persist", bufs=1) as persist,
            tc.tile_pool(name="psum", bufs=2, space="PSUM") as psum_pool,
            tc.tile_pool(name="mids", bufs=6) as mids,
        ):
            g_tiles = [
                persist.tile([128, 2, N_CODE], mybir.dt.float8e4, name=f"gt{kp}", tag=f"g{kp}")
                for kp in range(KP)
            ]
            b_groups = [
                persist.tile([128, BG, KS, MCW], mybir.dt.float8e4, name=f"bg{i}", tag=f"bg{i}")
                for i in range(NBG)
            ]

            # --- input pushes (order matters; each costs ~0.7us of the
            # issuing sequencer). scalar gets NONE so ACT starts on time.
            # sync: b0, g0-half, g1, g3, bgroup2, bgroup4
            # SWDGE: g0-half, g2, rest of bgroup1, bgroup3
            nc.sync.dma_start(out=b_groups[0][:, 0:1], in_=bt[:, 0:1, :, :])
            nc.gpsimd.dma_start(out=g_tiles[0][:, :, NH:], in_=g[:, 0:2, NH:])
            nc.sync.dma_start(out=g_tiles[0][:, :, 0:NH], in_=g[:, 0:2, 0:NH])
            nc.sync.dma_start(out=g_tiles[1], in_=g[:, 2:4, :])
            nc.gpsimd.dma_start(out=g_tiles[2], in_=g[:, 4:6, :])
            nc.sync.dma_start(out=g_tiles[3], in_=g[:, 6:8, :])
            nc.gpsimd.dma_start(out=b_groups[0][:, 1:BG], in_=bt[:, 1:BG, :, :])
            nc.sync.dma_start(out=b_groups[1], in_=bt[:, BG : 2 * BG, :, :])
            nc.gpsimd.dma_start(out=b_groups[2], in_=bt[:, 2 * BG : 3 * BG, :, :])
            nc.sync.dma_start(out=b_groups[3], in_=bt[:, 3 * BG : 4 * BG, :, :])

            # PE p-state pre-warm with dummy matmuls while queues start up
            zb = persist.tile([128, 2, 128], mybir.dt.float8e4, name="zwarm")
            nc.vector.memset(zb, 0)
            ps_warm = psum_pool.tile([128, N_CODE], mybir.dt.float32, name="ps")
            for w in range(18):
                nc.tensor.matmul(
                    ps_warm[:, 0:128], zb, zb, start=True, stop=True, perf_mode=dr
                )

            # output viewed per m-tile: m = mt*128 + p
            c_view = c.rearrange("(mt p) n -> mt p n", p=128)

            # out-DMA pushes alternate scalar (between its ACTs) and SWDGE
            out_eng = [(nc.gpsimd, nc.scalar)[i % 2] for i in range(MT)]

            def bsta(mt, kp):
                mc, j = mt // 2, mt % 2
                return b_groups[mc // BG][
                    :, mc % BG, 2 * kp : 2 * kp + 2, j * 128 : (j + 1) * 128
                ]

            def mm(ps, mt, kp, nt):
                nc.tensor.matmul(
                    ps[:, nt * 512 : (nt + 1) * 512],
                    bsta(mt, kp),
                    g_tiles[kp][:, :, nt * 512 : (nt + 1) * 512],
                    start=(kp == 0),
                    stop=(kp == KP - 1),
                    perf_mode=dr,
                )

            for mt in range(MT):
                if mt < MT - 1:
                    ps = psum_pool.tile([128, N_CODE], mybir.dt.float32, name="ps")
                    for kp in range(KP):
                        for nt in range(NT):
                            mm(ps, mt, kp, nt)
                    mid = mids.tile([128, N_CODE], mybir.dt.uint16)
                    nc.scalar.activation(mid, ps, mybir.ActivationFunctionType.Copy)
                    nc.vector.tensor_scalar(
                        out=mid,
                        in0=mid,
                        scalar1=1,
                        scalar2=None,
                        op0=mybir.AluOpType.bitwise_and,
                    )
                    out_eng[mt].dma_start(out=c_view[mt], in_=mid)
                else:
                    # last tile: per-bank PSUM tiles, extract + stream out
                    # each 512-col quarter while the PE finishes the rest
                    mid = mids.tile([128, N_CODE], mybir.dt.uint16)
                    for nt in range(NT):
                        psq = psum_pool.tile([128, 512], mybir.dt.float32, name="ps")
                        for kp in range(KP):
                            nc.tensor.matmul(
                                psq,
                                bsta(mt, kp),
                                g_tiles[kp][:, :, nt * 512 : (nt + 1) * 512],
                                start=(kp == 0),
                                stop=(kp == KP - 1),
                                perf_mode=dr,
                            )
                        nc.scalar.activation(
                            mid[:, nt * 512 : (nt + 1) * 512],
                            psq,
                            mybir.ActivationFunctionType.Copy,
                        )
                        nc.vector.tensor_scalar(
                            out=mid[:, nt * 512 : (nt + 1) * 512],
                            in0=mid[:, nt * 512 : (nt + 1) * 512],
                            scalar1=1,
                            scalar2=None,
                            op0=mybir.AluOpType.bitwise_and,
                        )
                        out_eng[mt].dma_start(
                            out=c_view[mt][:, nt * 512 : (nt + 1) * 512],
                            in_=mid[:, nt * 512 : (nt + 1) * 512],
                        )

    nc.finalize()
    return nc


def _get_nc():
    global _NC_CACHE
    if _NC_CACHE is None:
        _NC_CACHE = _build_bass()
    return _NC_CACHE


def _pack_inputs(b, G):
    b8 = np.asarray(b).astype(np.uint8)
    G8 = np.asarray(G).astype(np.uint8)
    # g[p, s, n], k = s*128 + p
    g_f8 = G8.reshape(KS, 128, N_CODE).transpose(1, 0, 2).astype(F8, order="C")
    bts = []
    for core in range(NCORES):
        sh = b8[core * M : (core + 1) * M]  # [M, K]
        # bt[p, c, s, j]: m = c*MCW + j, k = s*128 + p
        btc = sh.reshape(MC, MCW, KS, 128).transpose(3, 0, 2, 1)
        bts.append(btc.astype(F8, order="C"))
    return bts, g_f8


def kernel(b, G, trace=False, **run_kwargs):
    from concourse.bass_utils import run_bass_kernel_spmd

    nc = _get_nc()
    bts, g_f8 = _pack_inputs(b, G)
    in_maps = [{"bt": bts[i], "g": g_f8} for i in range(NCORES)]
    res = run_bass_kernel_spmd(
        nc, in_maps, core_ids=list(range(NCORES)), trace=trace, **run_kwargs
    )
    out = np.concatenate([res.results[i]["c"] for i in range(NCORES)], axis=0)
    out = out.astype(np.int32)
    if trace:
        kernel.last_results = res
    return out


kernel.last_results = None
